# revision 11
# baseline (speedup 1.0000x reference)
"""Causal self-attention (B=2, S=2048, D=1024, H=16) on 8 Trainium2 cores.

Sharding: core c handles batch b = c // 4 and head group g = c % 4
(4 heads = 256 embedding columns). Fully SPMD: one Bass program, per-core
input maps.

v6 pipeline (per core):
  - Q/K/V projections in fp32r (1 cyc/col, at roofline ~48us measured).
  - scores: per (head, chunk) [128 j, 512 i] fp32r matmuls into SEPARATE
    per-head PSUM tiles (st_a / st_b, 1 bank each) so the two heads of a
    pair carry disjoint tile_positions (0,0)/(64,0) (K=64 row strips)
    and can overlap on the PE; emitted back-to-back per chunk.
  - exp on ACT per (head, chunk): N=512-off, ~(N+48)/1.2 ns, f32r pt.
    (fp8 probs/V were tried and REJECTED: o is a weighted mean of V, so
    the signal shrinks by the same sqrt(n_eff) as quantization noise --
    any per-element eps on probs/V lands ~1:1 on output rel err;
    fp8e4 measured 3.3e-2 > the 2e-2 gate.  f32r is same PE speed.)
  - causal mask + pad zero-fill in one gpsimd affine_select per diagonal
    chunk over [mm, off+128): predicate col-off >= p.
  - PV: per (head, chunk) [65, 512] f32r matmul, vaug[128, 65] = V plus
    ones column, accumulating [O^T; den] in PSUM o_a/o_b over chunks.
  - projections woven into the chunk stream by a deadline-driven
    scheduler (producers forced before first consumer, ~1 unit/chunk).
  - host divides [O^T; den], transposes, assembles (unchanged).

kernel() hardening: the first device invocation after load can return
silently corrupted finite results (observed ~7e-2 rel err, later runs
exact); kernel() always discards a warmup invocation, then validates a
deterministic sample of output rows against numpy and retries.
"""

import numpy as np

B, S, D, H = 2, 2048, 1024, 16
HD = D // H          # 64
NCORES = 8
GROUPS = 4           # head groups (cores per batch)
HPC = H // GROUPS    # heads per core = 4
EC = HPC * HD        # e-columns per core = 256
DCH = D // 128       # d chunks = 8
JCH = S // 128       # j chunks = 16
IH_W = 1024          # i-half width

_NC_CACHE = {}


class _Emitter:
    """Emission helpers bound to one Bass/TileContext build."""

    def __init__(self, nc, tc, pp, ptp, ps, dram):
        import concourse.mybir as mybir
        self.mybir = mybir
        self.nc, self.tc, self.pp, self.ptp, self.ps = nc, tc, pp, ptp, ps
        self.f32 = mybir.dt.float32
        self.f32r = mybir.dt.float32r
        self.fp8 = mybir.dt.float8e4
        self.AF = mybir.ActivationFunctionType
        (self.xP, self.wP, self.out) = dram
        self.scale = 1.0 / np.sqrt(HD)

    # ---------- staging ----------
    def stage(self):
        nc, pp = self.nc, self.pp
        f32r = self.f32r
        # one weight tile [128, 3(qkv), 8(k-chunk), EC] and four per-slab x
        # tiles [128, 8(k-chunk), 512]; 5 large DMAs total, each with 128
        # contiguous >=8KB descriptors
        self.wt = pp.tile([128, 3, DCH, EC], f32r, tag="wt", name="wt")
        self.xs = [pp.tile([128, DCH, 512], f32r, tag=f"xs{sg}",
                           name=f"xs{sg}") for sg in range(4)]
        nc.sync.dma_start(out=self.wt[:], in_=self.wP[:].bitcast(f32r))
        for sg in range(4):
            nc.sync.dma_start(out=self.xs[sg][:],
                              in_=self.xP[sg].bitcast(f32r))

        self.qt = [[pp.tile([128, IH_W], self.f32r, tag=f"qt{m}{ih}",
                            name=f"qt{m}{ih}") for ih in range(2)]
                   for m in range(2)]
        self.kt = [[pp.tile([128, IH_W], self.f32r, tag=f"kt{m}{jh}",
                            name=f"kt{m}{jh}") for jh in range(2)]
                   for m in range(2)]
        # V for chunk j: [128 k-row, 4 head, 65] f32r (col 64 = ones for
        # the denominator row)
        self.vaug = [pp.tile([128, HPC, 65], self.f32r, tag=f"va{j}",
                             name=f"va{j}") for j in range(JCH)]
        # ones columns: f32 memset const, DVE copies (f32 -> f32r)
        cst = pp.tile([128, HPC], self.f32, tag="cst", name="cst")
        nc.vector.memset(cst[:], 1.0)
        for j in range(JCH):
            nc.vector.tensor_copy(self.vaug[j][:, :, 64:65], cst[:])

    def warmup(self, n=10):
        """Dummy fp32 matmuls: keep the PE busy during input staging so it
        reaches the 2.4 GHz p-state before real work arrives."""
        nc, pp, ps = self.nc, self.pp, self.ps
        wt = pp.tile([128, 577], self.f32, tag="warm", name="warm")
        nc.vector.memset(wt[:], 0.0)
        pw = ps.tile([65, 512], self.f32, tag="oa", bufs=1, name="pwarm")
        for _ in range(n):
            nc.tensor.matmul(pw[:], wt[:, 0:65], wt[:, 65:577],
                             start=True, stop=True)

    # ---------- fp32r projections ----------
    def qk_tile(self, name, m, sg, half=None):
        """One [128, 512] tile of Q^T or K^T: pair m, s-slab sg.
        half=0/1 computes only 256 output columns (smaller weave unit)."""
        nc, ps = self.nc, self.ps
        wi = {"q": 0, "k": 1}[name]
        dest = (self.qt if name == "q" else self.kt)[m][sg // 2]
        off = 512 * (sg % 2)
        ml = slice(128 * m, 128 * m + 128)
        xs = self.xs[sg]
        if half is None:
            cs, W = 0, 512
        else:
            cs, W = 256 * half, 256
        pq = ps.tile([128, W], self.f32, tag="stf", bufs=2, name="pq")
        for k in range(DCH):
            nc.tensor.matmul(pq[:], self.wt[:, wi, k, ml],
                             xs[:, k, cs:cs + W],
                             start=(k == 0), stop=(k == DCH - 1))
        nc.vector.tensor_copy(dest[:, off + cs:off + cs + W], pq[:])

    def v_chunk(self, j):
        """V for j-chunk j -> vaug[j][:, :, 0:64] (natural [S, EC] layout)."""
        nc, ps = self.nc, self.ps
        xs = self.xs[j // 4]
        jl = slice(128 * (j % 4), 128 * (j % 4) + 128)
        pv = ps.tile([128, EC], self.f32, tag="stf", bufs=2, name="pv")
        for k in range(DCH):
            nc.tensor.matmul(pv[:], xs[:, k, jl], self.wt[:, 2, k],
                             start=(k == 0), stop=(k == DCH - 1))
        nc.vector.tensor_copy(
            self.vaug[j][:, :, 0:64],
            pv[:].rearrange("p (h x) -> p h x", h=HPC))

    # ---------- attention (fp32r, batched scores / batched PV) ----------
    # Microbenched: K=64 head-pair score matmuls on disjoint row strips
    # (tile_position (0,0)/(64,0)) run ~2-3x concurrent when emitted
    # back-to-back (63-101 ns/MM vs 213 serial), but any interleaved
    # K=128 matmul (PV) serializes the array (~1133 ns/chunk).  So per
    # B-chunk group: all score pairs first, then all PV matmuls (lagged
    # one group behind through the exp).
    BATCH = 2

    def scores_chunk(self, p, it, c):
        """Both heads' scores for chunk c into one [128,1024] st tile
        (head a cols 0-511, head b 512-1023).  Returns (st, off, mm)."""
        nc, ps = self.nc, self.ps
        i0 = 512 * it
        off = max(0, 128 * c - i0)
        mm = min(off, 256)          # keep matmul width >= 256 (f32r rate)
        jh, joff = c // 8, 128 * c - IH_W * (c // 8)
        qoff = 512 * (it % 2)
        st = ps.tile([128, IH_W], self.f32, tag="sta", bufs=2, name="st")
        for hh in (0, 1):
            lo, hi = 64 * hh, 64 * hh + 64
            nc.tensor.matmul(
                st[:, 512 * hh + mm:512 * hh + 512],
                self.kt[p][jh][lo:hi, joff:joff + 128],
                self.qt[p][it // 2][lo:hi, qoff + mm:qoff + 512],
                start=True, stop=True)
        return (st, off, mm)

    def exp_chunk(self, p, it, c, sc):
        """One exp over both heads; affine_select masks diagonal chunks.
        Returns (pt, mm)."""
        nc, ptp, mybir = self.nc, self.ptp, self.mybir
        st, off, mm = sc
        i0 = 512 * it
        pt = ptp.tile([128, IH_W], self.f32r, tag="pt", name="pt")
        if off == 0:
            nc.scalar.activation(pt[:], st[:], self.AF.Exp, scale=self.scale)
        else:
            nc.scalar.activation(
                pt[:].rearrange("q (h w) -> q h w", h=2)[:, :, off:512],
                st[:].rearrange("q (h w) -> q h w", h=2)[:, :, off:512],
                self.AF.Exp, scale=self.scale)
        if 128 * c >= i0:
            w = off - mm + 128
            for hh in (0, 1):
                r = pt[:, 512 * hh + mm:512 * hh + mm + w]
                nc.gpsimd.affine_select(
                    out=r, in_=r,
                    compare_op=mybir.AluOpType.is_ge,
                    fill=0.0, base=mm - off,
                    pattern=[[1, w]], channel_multiplier=-1)
        return (pt, mm)

    def pv_chunk(self, p, it, c, o_a, o_b, pv):
        nc = self.nc
        pt, mm = pv
        last = (c == 4 * it + 3)
        for hh, o_t in ((0, o_a), (1, o_b)):
            nc.tensor.matmul(
                o_t[:, mm:512], self.vaug[c][:, 2 * p + hh, :],
                pt[:, 512 * hh + mm:512 * hh + 512],
                start=(c == 0), stop=last)

    def attn_segment(self, p, it, weaver=None):
        """One (head-pair, 512-wide i-tile) segment; chunks 0..4it+3 in
        groups of BATCH: score pairs back-to-back, then previous group's
        PV matmuls, fillers in between."""
        nc, ps, ptp = self.nc, self.ps, self.ptp
        seg = 4 * p + it
        B = self.BATCH
        o_a = ps.tile([65, 512], self.f32, tag="oa", bufs=1, name="o_a")
        o_b = ps.tile([65, 512], self.f32, tag="ob", bufs=1, name="o_b")
        cmax = 4 * it + 4
        prev = []
        for c0 in range(0, cmax, B):
            if weaver is not None:
                weaver(seg, c0 + B - 1, "pre")
            group = [self.scores_chunk(p, it, c) for c in range(c0, c0 + B)]
            pvs = [self.exp_chunk(p, it, c0 + e, sc)
                   for e, sc in enumerate(group)]
            if weaver is not None:
                weaver(seg, c0 + B - 1, "mid")
            for e, pv in enumerate(prev):
                self.pv_chunk(p, it, c0 - B + e, o_a, o_b, pv)
            prev = pvs
        for e, pv in enumerate(prev):
            self.pv_chunk(p, it, cmax - B + e, o_a, o_b, pv)
        for hh, o_t in ((0, o_a), (1, o_b)):
            o_sb = ptp.tile([65, 512], self.f32, tag="osb", bufs=4, name="o_sb")
            nc.vector.tensor_copy(o_sb[:], o_t[:])
            nc.sync.dma_start(out=self.out[2 * p + hh, it], in_=o_sb[:])

    # ---------- schedules ----------
    def qkv_all(self):
        for sg in range(4):
            for m in range(2):
                self.qk_tile("q", m, sg)
                self.qk_tile("k", m, sg)
        for j in range(JCH):
            self.v_chunk(j)

    def attn_all(self, weaver=None):
        for p in range(2):
            for it in range(4):
                self.attn_segment(p, it, weaver)

    def _units(self):
        """Projection units with (segment, chunk) deadlines."""
        units = []
        for m in range(2):
            for sg in range(4):
                seg = 4 * m + sg
                for h in range(2):
                    units.append(((seg, 0), lambda m=m, sg=sg, h=h:
                                  self.qk_tile("q", m, sg, half=h)))
                    units.append(((seg, 4 * sg + 2 * h), lambda m=m, sg=sg, h=h:
                                  self.qk_tile("k", m, sg, half=h)))
        for j in range(JCH):
            it0 = j // 4                  # first i-tile with 4*it+3 >= j
            units.append(((it0, j), lambda j=j: self.v_chunk(j)))
        units.sort(key=lambda u: u[0])
        return units

    def schedule_v6(self):
        units = self._units()
        state = {"i": 0}

        def weaver(seg, c, phase):
            if phase == "pre":
                # producers due at this chunk: must precede its consumers
                while state["i"] < len(units):
                    dl, fn = units[state["i"]]
                    if dl <= (seg, c):
                        fn(); state["i"] += 1
                    else:
                        break
            else:
                # one voluntary filler + anything due by the next slot
                n_vol = 1
                while state["i"] < len(units):
                    dl, fn = units[state["i"]]
                    if dl <= (seg, c + 1):
                        fn(); state["i"] += 1
                    elif n_vol > 0:
                        fn(); state["i"] += 1; n_vol -= 1
                    else:
                        break

        # pre-phase: minimum inputs for segment 0 pair 0
        while state["i"] < len(units):
            dl, fn = units[state["i"]]
            if dl <= (0, 0):
                fn(); state["i"] += 1
            else:
                break
        self.attn_all(weaver)
        # any stragglers (shouldn't happen)
        while state["i"] < len(units):
            units[state["i"]][1](); state["i"] += 1

    # ---------- diagnostic-only helpers (not used by the graded path) ----
    def stage_ptfix(self):
        nc, pp = self.nc, self.pp
        tmp = pp.tile([128, IH_W], self.f32, tag="ptf0", name="ptf0")
        nc.vector.memset(tmp[:], 0.5)
        self.ptfix = pp.tile([128, IH_W], self.f32r, tag="ptfix", name="ptfix")
        nc.vector.tensor_copy(self.ptfix[:], tmp[:])

    def attn_noexp(self):
        """Batched scores + PV matmuls only; PV reads a fixed pt."""
        nc, ps = self.nc, self.ps
        B = self.BATCH
        for p in range(2):
            for it in range(4):
                o_a = ps.tile([65, 512], self.f32, tag="oa", bufs=1, name="o_a")
                o_b = ps.tile([65, 512], self.f32, tag="ob", bufs=1, name="o_b")
                cmax = 4 * it + 4
                prev = []
                for c0 in range(0, cmax, B):
                    group = [self.scores_chunk(p, it, c)
                             for c in range(c0, c0 + B)]
                    for e, (st, off, mm) in enumerate(prev):
                        c = c0 - B + e
                        last = (c == cmax - 1)
                        for hh, o_t in ((0, o_a), (1, o_b)):
                            nc.tensor.matmul(
                                o_t[:, mm:512], self.vaug[c][:, 2 * p + hh, :],
                                self.ptfix[:, 512 * hh + mm:512 * hh + 512],
                                start=(c == 0), stop=last)
                    prev = group
                for e, (st, off, mm) in enumerate(prev):
                    c = cmax - B + e
                    last = (c == cmax - 1)
                    for hh, o_t in ((0, o_a), (1, o_b)):
                        nc.tensor.matmul(
                            o_t[:, mm:512], self.vaug[c][:, 2 * p + hh, :],
                            self.ptfix[:, 512 * hh + mm:512 * hh + 512],
                            start=(c == 0), stop=last)
                for hh, o_t in ((0, o_a), (1, o_b)):
                    o_sb = self.ptp.tile([65, 512], self.f32, tag="osb",
                                         bufs=4, name="o_sb")
                    nc.vector.tensor_copy(o_sb[:], o_t[:])
                    nc.sync.dma_start(out=self.out[2 * p + hh, it], in_=o_sb[:])

    def actbench(self, n=40):
        """n back-to-back 1024-col exps (PSUM->SBUF), no other work."""
        nc, ps, ptp = self.nc, self.ps, self.ptp
        st = ps.tile([128, IH_W], self.f32, tag="sta", bufs=1, name="stb")
        nc.tensor.matmul(st[:, 0:512], self.wt[:, 0, 0, 0:128],
                         self.xs[0][:, 0], start=True, stop=True)
        nc.tensor.matmul(st[:, 512:1024], self.wt[:, 0, 0, 0:128],
                         self.xs[0][:, 1], start=True, stop=True)
        for _ in range(n):
            pt = ptp.tile([128, IH_W], self.f32r, tag="pt", name="ptb")
            nc.scalar.activation(pt[:], st[:], self.AF.Exp, scale=self.scale)

    def outputs_stub(self):
        nc, ptp = self.nc, self.ptp
        for ph in range(HPC):
            for it in range(4):
                z = ptp.tile([65, 512], self.f32, tag="osb", bufs=4, name="z")
                nc.gpsimd.memset(z[:], 0.0)
                nc.sync.dma_start(out=self.out[ph, it], in_=z[:])


def _build_nc(repeat=1, mode="full"):
    import concourse.bacc as bacc
    import concourse.mybir as mybir
    from concourse.tile import TileContext

    f32 = mybir.dt.float32

    nc = bacc.Bacc("TRN2", target_bir_lowering=False, debug=False)

    # x: [slab, 128 p(d%128), 8 k-chunk, 512 s]; w: [128 p, 3 (q/k/v), 8, EC]
    x = nc.declare_dram_parameter("x", [4, 128, DCH, 512], f32, isOutput=False)
    w = nc.declare_dram_parameter("w", [128, 3, DCH, EC], f32, isOutput=False)
    # per head-local, i-tile: [O^T rows 0..63 ; den row 64] x 512 i
    out = nc.declare_dram_parameter("o", [HPC, 4, 65, 512], f32, isOutput=True)
    dram = (x, w, out)

    with TileContext(nc) as tc, (
        tc.tile_pool(name="persist", bufs=1)) as pp, (
        tc.tile_pool(name="pt", bufs=6)) as ptp, (
        tc.tile_pool(name="ps", bufs=1, space="PSUM")) as ps:
        em = _Emitter(nc, tc, pp, ptp, ps, dram)

        def loop(body):
            if repeat == 1:
                body()
            else:
                with tc.For_i(0, repeat, 1, hint_engines=(mybir.EngineType.PE,)):
                    body()

        if mode == "full":
            em.warmup(10)
            def body():
                em.stage()
                em.schedule_v6()
            loop(body)
        elif mode == "serial":
            em.warmup(10)
            def body():
                em.stage()
                em.qkv_all()
                em.attn_all()
            loop(body)
        elif mode == "qkv":
            em.stage()
            loop(em.qkv_all)
            em.outputs_stub()
        elif mode == "attn":
            em.stage()
            em.qkv_all()
            loop(em.attn_all)
        elif mode == "dma":
            def body():
                em.stage()
                em.outputs_stub()
            loop(body)
        elif mode == "attn_noexp":
            em.stage()
            em.stage_ptfix()
            em.qkv_all()
            loop(em.attn_noexp)
        elif mode == "actbench":
            em.stage()
            loop(lambda: em.actbench(40))
            em.outputs_stub()
        else:
            raise ValueError(mode)

    nc.compile()
    return nc


def _get_nc():
    if "nc" not in _NC_CACHE:
        _NC_CACHE["nc"] = _build_nc()
    return _NC_CACHE["nc"]


def _numpy_fallback(hidden_states, attention_mask, Wq, bq, Wk, bk, Wv, bv):
    hs = np.asarray(hidden_states, np.float64)
    b, s, d = hs.shape

    def proj(W, bias):
        y = hs @ np.asarray(W, np.float64).T + np.asarray(bias, np.float64)
        return y.reshape(b, s, H, HD).transpose(0, 2, 1, 3)

    q, k, v = proj(Wq, bq), proj(Wk, bk), proj(Wv, bv)
    scores = np.einsum("bhqd,bhkd->bhqk", q, k) / np.sqrt(HD)
    causal = np.tril(np.ones((s, s), bool))[None, None]
    pad = ~(np.asarray(attention_mask).astype(bool))
    mask = causal & pad
    scores = np.where(mask, scores, -np.inf)
    scores -= scores.max(axis=-1, keepdims=True)
    e = np.exp(scores)
    probs = e / e.sum(axis=-1, keepdims=True)
    o = np.einsum("bhqk,bhkd->bhqd", probs, v)
    return o.transpose(0, 2, 1, 3).reshape(b, s, d).astype(np.float32)


def _make_in_maps(hs, Wq, Wk, Wv):
    in_maps = []
    xcache = {}
    # W.T [D, D]: -> [k, 128, out] -> [128, k, out]
    wT = {n: np.ascontiguousarray(W.T).reshape(DCH, 128, D).transpose(1, 0, 2)
          for n, W in (("q", Wq), ("k", Wk), ("v", Wv))}
    for c in range(NCORES):
        b, g = c // GROUPS, c % GROUPS
        if b not in xcache:
            # xT [D, S] -> [k, 128, slab, 512] -> [slab, 128, k, 512]
            a = np.ascontiguousarray(hs[b].T).reshape(DCH, 128, 4, 512)
            xcache[b] = np.ascontiguousarray(a.transpose(2, 1, 0, 3))
        sl = slice(EC * g, EC * g + EC)
        w = np.stack([wT["q"][:, :, sl], wT["k"][:, :, sl],
                      wT["v"][:, :, sl]], axis=1)     # [128, 3, k, EC]
        in_maps.append({"x": xcache[b], "w": np.ascontiguousarray(w)})
    return in_maps


def _assemble(results):
    out = np.empty((B, S, D), np.float32)
    ok = True
    for c in range(NCORES):
        arr = results[c]["o"]                    # [4, 4, 65, 512]
        o = arr[:, :, :64, :]
        den = arr[:, :, 64:65, :]
        if not np.all(np.isfinite(den)) or np.any(den <= 0.0):
            ok = False
        with np.errstate(all="ignore"):
            oh = o / den                         # [4, 4, 64, 512]
        oh = np.concatenate([oh[:, t] for t in range(4)], axis=-1)  # [4, 64, 2048]
        b, g = c // GROUPS, c % GROUPS
        out[b, :, EC * g:EC * g + EC] = oh.transpose(2, 0, 1).reshape(S, EC)
    return out, ok


_CHECK_ROWS = [0, 1, 63, 64, 127, 255, 256, 511, 512, 1023, 1024, 1535, 2047]


def _sample_check(out, hs, Wq, Wk, Wv):
    """Rel error of a deterministic sample of output rows vs numpy fp32."""
    err_n = 0.0
    ref_n = 0.0
    for b in range(B):
        x = hs[b]                                   # [S, D]
        K = x @ Wk.T                                # [S, D]
        V = x @ Wv.T
        for i in _CHECK_ROWS:
            q = x[i] @ Wq.T                         # [D]
            qh = q.reshape(H, HD)
            kh = K[:i + 1].reshape(i + 1, H, HD)
            vh = V[:i + 1].reshape(i + 1, H, HD)
            s = np.einsum("hd,jhd->hj", qh, kh) / np.sqrt(HD)
            s -= s.max(axis=1, keepdims=True)
            e = np.exp(s)
            p = e / e.sum(axis=1, keepdims=True)
            o = np.einsum("hj,jhd->hd", p, vh).reshape(D)
            d = out[b, i].astype(np.float64) - o.astype(np.float64)
            err_n += float(d @ d)
            ref_n += float(o @ o)
    return np.sqrt(err_n / max(ref_n, 1e-30))


def kernel(hidden_states, attention_mask, Wq, bq, Wk, bk, Wv, bv):
    from concourse.bass_utils import run_bass_kernel_spmd

    hs = np.asarray(hidden_states, np.float32)
    Wq = np.asarray(Wq, np.float32)
    Wk = np.asarray(Wk, np.float32)
    Wv = np.asarray(Wv, np.float32)

    # device path assumes the harness defaults: all-valid mask, zero biases
    if (np.any(np.asarray(attention_mask) != 0)
            or np.any(np.asarray(bq)) or np.any(np.asarray(bk)) or np.any(np.asarray(bv))
            or hs.shape != (B, S, D)):
        return _numpy_fallback(hidden_states, attention_mask, Wq, bq, Wk, bk, Wv, bv)

    nc = _get_nc()
    in_maps = _make_in_maps(hs, Wq, Wk, Wv)

    # warmup invocation: the first NEFF execution after load has been
    # observed to return silently corrupted (finite) results; discard it.
    run_bass_kernel_spmd(nc, in_maps, core_ids=list(range(NCORES)))

    out = None
    for _attempt in range(4):
        res = run_bass_kernel_spmd(nc, in_maps, core_ids=list(range(NCORES)))
        out, ok = _assemble(res.results)
        if ok and np.all(np.isfinite(out)):
            if _sample_check(out, hs, Wq, Wk, Wv) < 1.2e-2:
                return out
    return out


# revision 14
# speedup vs baseline: 1.6148x; 1.6148x over previous
"""Causal self-attention (B=2, S=2048, D=1024, H=16) on 8 Trainium2 cores.

Sharding: core c handles batch b = c // 4 and head group g = c % 4
(4 heads = 256 embedding columns). Fully SPMD: one Bass program, per-core
input maps.

v6 pipeline (per core):
  - Q/K/V projections in fp32r (1 cyc/col, at roofline ~48us measured).
  - scores: per (head, chunk) [128 j, 512 i] fp32r matmuls into SEPARATE
    per-head PSUM tiles (st_a / st_b, 1 bank each) so the two heads of a
    pair carry disjoint tile_positions (0,0)/(64,0) (K=64 row strips)
    and can overlap on the PE; emitted back-to-back per chunk.
  - exp on ACT per (head, chunk): N=512-off, ~(N+48)/1.2 ns, f32r pt.
    (fp8 probs/V were tried and REJECTED: o is a weighted mean of V, so
    the signal shrinks by the same sqrt(n_eff) as quantization noise --
    any per-element eps on probs/V lands ~1:1 on output rel err;
    fp8e4 measured 3.3e-2 > the 2e-2 gate.  f32r is same PE speed.)
  - causal mask + pad zero-fill in one gpsimd affine_select per diagonal
    chunk over [mm, off+128): predicate col-off >= p.
  - PV: per (head, chunk) [65, 512] f32r matmul, vaug[128, 65] = V plus
    ones column, accumulating [O^T; den] in PSUM o_a/o_b over chunks.
  - projections woven into the chunk stream by a deadline-driven
    scheduler (producers forced before first consumer, ~1 unit/chunk).
  - host divides [O^T; den], transposes, assembles (unchanged).

kernel() hardening: the first device invocation after load can return
silently corrupted finite results (observed ~7e-2 rel err, later runs
exact); kernel() always discards a warmup invocation, then validates a
deterministic sample of output rows against numpy and retries.
"""

import numpy as np

B, S, D, H = 2, 2048, 1024, 16
HD = D // H          # 64
NCORES = 8
GROUPS = 4           # head groups (cores per batch)
HPC = H // GROUPS    # heads per core = 4
EC = HPC * HD        # e-columns per core = 256
DCH = D // 128       # d chunks = 8
JCH = S // 128       # j chunks = 16
IH_W = 1024          # i-half width

_NC_CACHE = {}


class _Emitter:
    """Emission helpers bound to one Bass/TileContext build."""

    def __init__(self, nc, tc, pp, ptp, ps, dram):
        import concourse.mybir as mybir
        self.mybir = mybir
        self.nc, self.tc, self.pp, self.ptp, self.ps = nc, tc, pp, ptp, ps
        self.f32 = mybir.dt.float32
        self.f32r = mybir.dt.float32r
        self.fp8 = mybir.dt.float8e4
        self.AF = mybir.ActivationFunctionType
        (self.xP, self.wP, self.out) = dram
        self.scale = 1.0 / np.sqrt(HD)

    # ---------- staging ----------
    def stage(self):
        nc, pp = self.nc, self.pp
        f32r = self.f32r
        # one weight tile [128, 3(qkv), 8(k-chunk), EC] and four per-slab x
        # tiles [128, 8(k-chunk), 512]; 5 large DMAs total, each with 128
        # contiguous >=8KB descriptors
        self.wt = pp.tile([128, 3, DCH, EC], f32r, tag="wt", name="wt")
        self.xs = [pp.tile([128, DCH, 512], f32r, tag=f"xs{sg}",
                           name=f"xs{sg}") for sg in range(4)]
        nc.sync.dma_start(out=self.wt[:], in_=self.wP[:].bitcast(f32r))
        for sg in range(4):
            nc.sync.dma_start(out=self.xs[sg][:],
                              in_=self.xP[sg].bitcast(f32r))

        self.qt = [[pp.tile([128, IH_W], self.f32r, tag=f"qt{m}{ih}",
                            name=f"qt{m}{ih}") for ih in range(2)]
                   for m in range(2)]
        self.kt = [[pp.tile([128, IH_W], self.f32r, tag=f"kt{m}{jh}",
                            name=f"kt{m}{jh}") for jh in range(2)]
                   for m in range(2)]
        # V for chunk j: [128 k-row, 4 head, 65] f32r (col 64 = ones for
        # the denominator row)
        self.vaug = [pp.tile([128, HPC, 65], self.f32r, tag=f"va{j}",
                             name=f"va{j}") for j in range(JCH)]
        # ones columns: f32 memset const, DVE copies (f32 -> f32r)
        cst = pp.tile([128, HPC], self.f32, tag="cst", name="cst")
        nc.vector.memset(cst[:], 1.0)
        for j in range(JCH):
            nc.vector.tensor_copy(self.vaug[j][:, :, 64:65], cst[:])

    def warmup(self, n=10):
        """Dummy fp32 matmuls: keep the PE busy during input staging so it
        reaches the 2.4 GHz p-state before real work arrives."""
        nc, pp, ps = self.nc, self.pp, self.ps
        wt = pp.tile([128, 577], self.f32, tag="warm", name="warm")
        nc.vector.memset(wt[:], 0.0)
        pw = ps.tile([65, 512], self.f32, tag="oa", bufs=1, name="pwarm")
        for _ in range(n):
            nc.tensor.matmul(pw[:], wt[:, 0:65], wt[:, 65:577],
                             start=True, stop=True)

    # ---------- fp32r projections ----------
    def qk_tile(self, name, m, sg, half=None):
        """One [128, 512] tile of Q^T or K^T: pair m, s-slab sg.
        half=0/1 computes only 256 output columns (smaller weave unit)."""
        nc, ps = self.nc, self.ps
        wi = {"q": 0, "k": 1}[name]
        dest = (self.qt if name == "q" else self.kt)[m][sg // 2]
        off = 512 * (sg % 2)
        ml = slice(128 * m, 128 * m + 128)
        xs = self.xs[sg]
        if half is None:
            cs, W = 0, 512
        else:
            cs, W = 256 * half, 256
        pq = ps.tile([128, W], self.f32, tag="sta", bufs=3, name="pq")
        for k in range(DCH):
            nc.tensor.matmul(pq[:], self.wt[:, wi, k, ml],
                             xs[:, k, cs:cs + W],
                             start=(k == 0), stop=(k == DCH - 1))
        nc.vector.tensor_copy(dest[:, off + cs:off + cs + W], pq[:])

    def v_chunk(self, j):
        """V for j-chunk j -> vaug[j][:, :, 0:64] (natural [S, EC] layout)."""
        nc, ps = self.nc, self.ps
        xs = self.xs[j // 4]
        jl = slice(128 * (j % 4), 128 * (j % 4) + 128)
        pv = ps.tile([128, EC], self.f32, tag="sta", bufs=3, name="pv")
        for k in range(DCH):
            nc.tensor.matmul(pv[:], xs[:, k, jl], self.wt[:, 2, k],
                             start=(k == 0), stop=(k == DCH - 1))
        nc.vector.tensor_copy(
            self.vaug[j][:, :, 0:64],
            pv[:].rearrange("p (h x) -> p h x", h=HPC))

    # ---------- attention (fp32r, batched scores / batched PV) ----------
    # Microbenched: K=64 head-pair score matmuls on disjoint row strips
    # (tile_position (0,0)/(64,0)) run ~2-3x concurrent when emitted
    # back-to-back (63-101 ns/MM vs 213 serial), but any interleaved
    # K=128 matmul (PV) serializes the array (~1133 ns/chunk).  So per
    # B-chunk group: all score pairs first, then all PV matmuls (lagged
    # one group behind through the exp).
    BATCH = 2

    def scores_chunk(self, p, it, c):
        """Both heads' scores for chunk c into one [128,1024] st tile
        (head a cols 0-511, head b 512-1023).  Returns (st, off, mm)."""
        nc, ps = self.nc, self.ps
        i0 = 512 * it
        off = max(0, 128 * c - i0)
        mm = min(off, 256)          # keep matmul width >= 256 (f32r rate)
        jh, joff = c // 8, 128 * c - IH_W * (c // 8)
        qoff = 512 * (it % 2)
        st = ps.tile([128, IH_W], self.f32, tag="sta", bufs=3, name="st")
        for hh in (0, 1):
            lo, hi = 64 * hh, 64 * hh + 64
            nc.tensor.matmul(
                st[:, 512 * hh + mm:512 * hh + 512],
                self.kt[p][jh][lo:hi, joff:joff + 128],
                self.qt[p][it // 2][lo:hi, qoff + mm:qoff + 512],
                start=True, stop=True)
        return (st, off, mm)

    def exp_chunk(self, p, it, c, sc):
        """One exp over both heads; affine_select masks diagonal chunks.
        Returns (pt, mm)."""
        nc, ptp, mybir = self.nc, self.ptp, self.mybir
        st, off, mm = sc
        i0 = 512 * it
        pt = ptp.tile([128, IH_W], self.f32r, tag="pt", name="pt")
        if off == 0:
            nc.scalar.activation(pt[:], st[:], self.AF.Exp, scale=self.scale)
        else:
            nc.scalar.activation(
                pt[:].rearrange("q (h w) -> q h w", h=2)[:, :, off:512],
                st[:].rearrange("q (h w) -> q h w", h=2)[:, :, off:512],
                self.AF.Exp, scale=self.scale)
        if 128 * c >= i0:
            w = off - mm + 128
            for hh in (0, 1):
                r = pt[:, 512 * hh + mm:512 * hh + mm + w]
                nc.gpsimd.affine_select(
                    out=r, in_=r,
                    compare_op=mybir.AluOpType.is_ge,
                    fill=0.0, base=mm - off,
                    pattern=[[1, w]], channel_multiplier=-1)
        return (pt, mm)

    def pv_chunk(self, p, it, c, o_a, o_b, pv):
        nc = self.nc
        pt, mm = pv
        last = (c == 4 * it + 3)
        for hh, o_t in ((0, o_a), (1, o_b)):
            nc.tensor.matmul(
                o_t[:, mm:512], self.vaug[c][:, 2 * p + hh, :],
                pt[:, 512 * hh + mm:512 * hh + 512],
                start=(c == 0), stop=last)

    def attn_segment(self, p, it, weaver=None):
        """One (head-pair, 512-wide i-tile) segment; chunks 0..4it+3 in
        groups of BATCH: score pairs back-to-back, then previous group's
        PV matmuls, fillers in between."""
        nc, ps, ptp = self.nc, self.ps, self.ptp
        seg = 4 * p + it
        B = self.BATCH
        o_a = ps.tile([65, 512], self.f32, tag="oa", bufs=1, name="o_a")
        o_b = ps.tile([65, 512], self.f32, tag="ob", bufs=1, name="o_b")
        cmax = 4 * it + 4
        prev = []
        for c0 in range(0, cmax, B):
            if weaver is not None:
                weaver(seg, c0 + B - 1, "pre")
            group = [self.scores_chunk(p, it, c) for c in range(c0, c0 + B)]
            pvs = [self.exp_chunk(p, it, c0 + e, sc)
                   for e, sc in enumerate(group)]
            if weaver is not None:
                weaver(seg, c0 + B - 1, "mid")
            for e, pv in enumerate(prev):
                self.pv_chunk(p, it, c0 - B + e, o_a, o_b, pv)
            prev = pvs
        for e, pv in enumerate(prev):
            self.pv_chunk(p, it, cmax - B + e, o_a, o_b, pv)
        for hh, o_t in ((0, o_a), (1, o_b)):
            o_sb = ptp.tile([65, 512], self.f32, tag="osb", bufs=4, name="o_sb")
            nc.vector.tensor_copy(o_sb[:], o_t[:])
            nc.sync.dma_start(out=self.out[2 * p + hh, it], in_=o_sb[:])

    # ---------- schedules ----------
    def qkv_all(self):
        for sg in range(4):
            for m in range(2):
                self.qk_tile("q", m, sg)
                self.qk_tile("k", m, sg)
        for j in range(JCH):
            self.v_chunk(j)

    def attn_all(self, weaver=None):
        for p in range(2):
            for it in range(4):
                self.attn_segment(p, it, weaver)

    def _units(self):
        """Projection units with (segment, chunk) deadlines."""
        units = []
        for m in range(2):
            for sg in range(4):
                seg = 4 * m + sg
                for h in range(2):
                    units.append(((seg, 0), lambda m=m, sg=sg, h=h:
                                  self.qk_tile("q", m, sg, half=h)))
                    units.append(((seg, 4 * sg + 2 * h), lambda m=m, sg=sg, h=h:
                                  self.qk_tile("k", m, sg, half=h)))
        for j in range(JCH):
            it0 = j // 4                  # first i-tile with 4*it+3 >= j
            units.append(((it0, j), lambda j=j: self.v_chunk(j)))
        units.sort(key=lambda u: u[0])
        return units

    def schedule_v6(self):
        units = self._units()
        state = {"i": 0}

        def weaver(seg, c, phase):
            if phase == "pre":
                # producers due at this chunk: must precede its consumers
                while state["i"] < len(units):
                    dl, fn = units[state["i"]]
                    if dl <= (seg, c):
                        fn(); state["i"] += 1
                    else:
                        break
            else:
                # one voluntary filler + anything due by the next slot
                n_vol = 1
                while state["i"] < len(units):
                    dl, fn = units[state["i"]]
                    if dl <= (seg, c + 1):
                        fn(); state["i"] += 1
                    elif n_vol > 0:
                        fn(); state["i"] += 1; n_vol -= 1
                    else:
                        break

        # pre-phase: minimum inputs for segment 0 pair 0
        while state["i"] < len(units):
            dl, fn = units[state["i"]]
            if dl <= (0, 0):
                fn(); state["i"] += 1
            else:
                break
        self.attn_all(weaver)
        # any stragglers (shouldn't happen)
        while state["i"] < len(units):
            units[state["i"]][1](); state["i"] += 1

    # ---------- diagnostic-only helpers (not used by the graded path) ----
    def stage_ptfix(self):
        nc, pp = self.nc, self.pp
        tmp = pp.tile([128, IH_W], self.f32, tag="ptf0", name="ptf0")
        nc.vector.memset(tmp[:], 0.5)
        self.ptfix = pp.tile([128, IH_W], self.f32r, tag="ptfix", name="ptfix")
        nc.vector.tensor_copy(self.ptfix[:], tmp[:])

    def attn_noexp(self):
        """Batched scores + PV matmuls only; PV reads a fixed pt."""
        nc, ps = self.nc, self.ps
        B = self.BATCH
        for p in range(2):
            for it in range(4):
                o_a = ps.tile([65, 512], self.f32, tag="oa", bufs=1, name="o_a")
                o_b = ps.tile([65, 512], self.f32, tag="ob", bufs=1, name="o_b")
                cmax = 4 * it + 4
                prev = []
                for c0 in range(0, cmax, B):
                    group = [self.scores_chunk(p, it, c)
                             for c in range(c0, c0 + B)]
                    for e, (st, off, mm) in enumerate(prev):
                        c = c0 - B + e
                        last = (c == cmax - 1)
                        for hh, o_t in ((0, o_a), (1, o_b)):
                            nc.tensor.matmul(
                                o_t[:, mm:512], self.vaug[c][:, 2 * p + hh, :],
                                self.ptfix[:, 512 * hh + mm:512 * hh + 512],
                                start=(c == 0), stop=last)
                    prev = group
                for e, (st, off, mm) in enumerate(prev):
                    c = cmax - B + e
                    last = (c == cmax - 1)
                    for hh, o_t in ((0, o_a), (1, o_b)):
                        nc.tensor.matmul(
                            o_t[:, mm:512], self.vaug[c][:, 2 * p + hh, :],
                            self.ptfix[:, 512 * hh + mm:512 * hh + 512],
                            start=(c == 0), stop=last)
                for hh, o_t in ((0, o_a), (1, o_b)):
                    o_sb = self.ptp.tile([65, 512], self.f32, tag="osb",
                                         bufs=4, name="o_sb")
                    nc.vector.tensor_copy(o_sb[:], o_t[:])
                    nc.sync.dma_start(out=self.out[2 * p + hh, it], in_=o_sb[:])

    def actbench(self, n=40):
        """n back-to-back 1024-col exps (PSUM->SBUF), no other work."""
        nc, ps, ptp = self.nc, self.ps, self.ptp
        st = ps.tile([128, IH_W], self.f32, tag="sta", bufs=1, name="stb")
        nc.tensor.matmul(st[:, 0:512], self.wt[:, 0, 0, 0:128],
                         self.xs[0][:, 0], start=True, stop=True)
        nc.tensor.matmul(st[:, 512:1024], self.wt[:, 0, 0, 0:128],
                         self.xs[0][:, 1], start=True, stop=True)
        for _ in range(n):
            pt = ptp.tile([128, IH_W], self.f32r, tag="pt", name="ptb")
            nc.scalar.activation(pt[:], st[:], self.AF.Exp, scale=self.scale)

    def outputs_stub(self):
        nc, ptp = self.nc, self.ptp
        for ph in range(HPC):
            for it in range(4):
                z = ptp.tile([65, 512], self.f32, tag="osb", bufs=4, name="z")
                nc.gpsimd.memset(z[:], 0.0)
                nc.sync.dma_start(out=self.out[ph, it], in_=z[:])


def _build_nc(repeat=1, mode="full"):
    import concourse.bacc as bacc
    import concourse.mybir as mybir
    from concourse.tile import TileContext

    f32 = mybir.dt.float32

    nc = bacc.Bacc("TRN2", target_bir_lowering=False, debug=False)

    # x: [slab, 128 p(d%128), 8 k-chunk, 512 s]; w: [128 p, 3 (q/k/v), 8, EC]
    x = nc.declare_dram_parameter("x", [4, 128, DCH, 512], f32, isOutput=False)
    w = nc.declare_dram_parameter("w", [128, 3, DCH, EC], f32, isOutput=False)
    # per head-local, i-tile: [O^T rows 0..63 ; den row 64] x 512 i
    out = nc.declare_dram_parameter("o", [HPC, 4, 65, 512], f32, isOutput=True)
    dram = (x, w, out)

    with TileContext(nc) as tc, (
        tc.tile_pool(name="persist", bufs=1)) as pp, (
        tc.tile_pool(name="pt", bufs=6)) as ptp, (
        tc.tile_pool(name="ps", bufs=1, space="PSUM")) as ps:
        em = _Emitter(nc, tc, pp, ptp, ps, dram)

        def loop(body):
            if repeat == 1:
                body()
            else:
                with tc.For_i(0, repeat, 1, hint_engines=(mybir.EngineType.PE,)):
                    body()

        if mode == "full":
            em.warmup(10)
            def body():
                em.stage()
                em.schedule_v6()
            loop(body)
        elif mode == "serial":
            em.warmup(10)
            def body():
                em.stage()
                em.qkv_all()
                em.attn_all()
            loop(body)
        elif mode == "qkv":
            em.stage()
            loop(em.qkv_all)
            em.outputs_stub()
        elif mode == "attn":
            em.stage()
            em.qkv_all()
            loop(em.attn_all)
        elif mode == "dma":
            def body():
                em.stage()
                em.outputs_stub()
            loop(body)
        elif mode == "attn_noexp":
            em.stage()
            em.stage_ptfix()
            em.qkv_all()
            loop(em.attn_noexp)
        elif mode == "actbench":
            em.stage()
            loop(lambda: em.actbench(40))
            em.outputs_stub()
        else:
            raise ValueError(mode)

    nc.compile()
    return nc


def _get_nc():
    if "nc" not in _NC_CACHE:
        _NC_CACHE["nc"] = _build_nc()
    return _NC_CACHE["nc"]


def _numpy_fallback(hidden_states, attention_mask, Wq, bq, Wk, bk, Wv, bv):
    hs = np.asarray(hidden_states, np.float64)
    b, s, d = hs.shape

    def proj(W, bias):
        y = hs @ np.asarray(W, np.float64).T + np.asarray(bias, np.float64)
        return y.reshape(b, s, H, HD).transpose(0, 2, 1, 3)

    q, k, v = proj(Wq, bq), proj(Wk, bk), proj(Wv, bv)
    scores = np.einsum("bhqd,bhkd->bhqk", q, k) / np.sqrt(HD)
    causal = np.tril(np.ones((s, s), bool))[None, None]
    pad = ~(np.asarray(attention_mask).astype(bool))
    mask = causal & pad
    scores = np.where(mask, scores, -np.inf)
    scores -= scores.max(axis=-1, keepdims=True)
    e = np.exp(scores)
    probs = e / e.sum(axis=-1, keepdims=True)
    o = np.einsum("bhqk,bhkd->bhqd", probs, v)
    return o.transpose(0, 2, 1, 3).reshape(b, s, d).astype(np.float32)


def _make_in_maps(hs, Wq, Wk, Wv):
    in_maps = []
    xcache = {}
    # W.T [D, D]: -> [k, 128, out] -> [128, k, out]
    wT = {n: np.ascontiguousarray(W.T).reshape(DCH, 128, D).transpose(1, 0, 2)
          for n, W in (("q", Wq), ("k", Wk), ("v", Wv))}
    for c in range(NCORES):
        b, g = c // GROUPS, c % GROUPS
        if b not in xcache:
            # xT [D, S] -> [k, 128, slab, 512] -> [slab, 128, k, 512]
            a = np.ascontiguousarray(hs[b].T).reshape(DCH, 128, 4, 512)
            xcache[b] = np.ascontiguousarray(a.transpose(2, 1, 0, 3))
        sl = slice(EC * g, EC * g + EC)
        w = np.stack([wT["q"][:, :, sl], wT["k"][:, :, sl],
                      wT["v"][:, :, sl]], axis=1)     # [128, 3, k, EC]
        in_maps.append({"x": xcache[b], "w": np.ascontiguousarray(w)})
    return in_maps


def _assemble(results):
    out = np.empty((B, S, D), np.float32)
    ok = True
    for c in range(NCORES):
        arr = results[c]["o"]                    # [4, 4, 65, 512]
        o = arr[:, :, :64, :]
        den = arr[:, :, 64:65, :]
        if not np.all(np.isfinite(den)) or np.any(den <= 0.0):
            ok = False
        with np.errstate(all="ignore"):
            oh = o / den                         # [4, 4, 64, 512]
        oh = np.concatenate([oh[:, t] for t in range(4)], axis=-1)  # [4, 64, 2048]
        b, g = c // GROUPS, c % GROUPS
        out[b, :, EC * g:EC * g + EC] = oh.transpose(2, 0, 1).reshape(S, EC)
    return out, ok


_CHECK_ROWS = [0, 1, 63, 64, 127, 255, 256, 511, 512, 1023, 1024, 1535, 2047]


def _sample_check(out, hs, Wq, Wk, Wv):
    """Rel error of a deterministic sample of output rows vs numpy fp32."""
    err_n = 0.0
    ref_n = 0.0
    for b in range(B):
        x = hs[b]                                   # [S, D]
        K = x @ Wk.T                                # [S, D]
        V = x @ Wv.T
        for i in _CHECK_ROWS:
            q = x[i] @ Wq.T                         # [D]
            qh = q.reshape(H, HD)
            kh = K[:i + 1].reshape(i + 1, H, HD)
            vh = V[:i + 1].reshape(i + 1, H, HD)
            s = np.einsum("hd,jhd->hj", qh, kh) / np.sqrt(HD)
            s -= s.max(axis=1, keepdims=True)
            e = np.exp(s)
            p = e / e.sum(axis=1, keepdims=True)
            o = np.einsum("hj,jhd->hd", p, vh).reshape(D)
            d = out[b, i].astype(np.float64) - o.astype(np.float64)
            err_n += float(d @ d)
            ref_n += float(o @ o)
    return np.sqrt(err_n / max(ref_n, 1e-30))


def kernel(hidden_states, attention_mask, Wq, bq, Wk, bk, Wv, bv):
    from concourse.bass_utils import run_bass_kernel_spmd

    hs = np.asarray(hidden_states, np.float32)
    Wq = np.asarray(Wq, np.float32)
    Wk = np.asarray(Wk, np.float32)
    Wv = np.asarray(Wv, np.float32)

    # device path assumes the harness defaults: all-valid mask, zero biases
    if (np.any(np.asarray(attention_mask) != 0)
            or np.any(np.asarray(bq)) or np.any(np.asarray(bk)) or np.any(np.asarray(bv))
            or hs.shape != (B, S, D)):
        return _numpy_fallback(hidden_states, attention_mask, Wq, bq, Wk, bk, Wv, bv)

    nc = _get_nc()
    in_maps = _make_in_maps(hs, Wq, Wk, Wv)

    # warmup invocation: the first NEFF execution after load has been
    # observed to return silently corrupted (finite) results; discard it.
    run_bass_kernel_spmd(nc, in_maps, core_ids=list(range(NCORES)))

    out = None
    for _attempt in range(4):
        res = run_bass_kernel_spmd(nc, in_maps, core_ids=list(range(NCORES)))
        out, ok = _assemble(res.results)
        if ok and np.all(np.isfinite(out)):
            if _sample_check(out, hs, Wq, Wk, Wv) < 1.2e-2:
                return out
    return out
